# revision 12
# baseline (speedup 1.0000x reference)
"""FAGCN message-passing kernel for 8 Trainium2 NeuronCores.

Strategy (edge-parallel via dst-ownership, v2):
  - Nodes are assigned to the 8 cores snake-wise in degree-sorted order, so
    every core owns ~N/8 nodes, ~E/8 edges, and sees the same degree profile
    (the compiled SPMD program is shared; only the index inputs differ).
  - Gate decomposition: tanh(Linear([h_dst, h_src])) = tanh(p1[dst] + p2[src] + b)
    with p1 = x @ w_dst, p2 = x @ w_src.
  - dst-side (own) scalars come from a host-sharded copy of the owned rows
    (xown, in tile order) -> p1b_own, norm_own per tile, no gather needed.
  - src-side: phase 1 builds a compact bf16 table [NPAD, 72] fully staged in
    SBUF: cols 0:64 = x (bf16), cols 64:68 = (p2, norm) as f32 (bitcast
    pairs), cols 68:72 pad.  One DMA (in two halves) writes it to DRAM.
    Rows are 144B, so phase-2 indirect gathers move ~half the bytes of the
    f32 variant.
  - Phase 2: per 128-node tile (dst-major, degree-sorted so slot padding is
    tiny), [P,1] indirect-DMA gathers pull table rows per slot column; the
    gate is ACT tanh with per-partition bias (p1b_own), aggregation is a
    bf16 DVE multiply + strided reduce (2x DVE rate).
    z[dst] = norm[dst] * sum_s tanh(p1b[dst] + p2[src]) * norm[src] * x[src].
"""

import os
import sys

sys.path.insert(0, "/opt/trn_rl_repo")

import numpy as np

P = 128
FA = 72  # bf16 elems per table row (144B): 64 x | 4 (=2 f32: p2, norm) | 4 pad
FA32 = FA // 2  # f32 view cols
COL_P2 = 32  # f32 col of p2
COL_NORM = 33  # f32 col of norm

LAST_RESULTS = None  # BassKernelResults of the most recent HW run (for profiling)


def _ceil_to(a, m):
    return ((a + m - 1) // m) * m


class Plan:
    pass


def _prep(x, gate_w, gate_b, src, dst, ncores=8):
    """Host-side sharding: shapes/constants + per-core input maps."""
    x = np.asarray(x, dtype=np.float32)
    gate_w = np.asarray(gate_w, dtype=np.float32)
    gate_b = np.asarray(gate_b, dtype=np.float32)
    src = np.asarray(src).astype(np.int64)
    dst = np.asarray(dst).astype(np.int64)

    N, D = x.shape
    assert D == 64
    E = src.shape[0]

    pl = Plan()
    pl.N, pl.D, pl.E, pl.ncores = N, D, E, ncores
    pl.NPAD = _ceil_to(N + 1, P)
    pl.CH = pl.NPAD // P
    # table row of node n under the partition-major layout r(n) = (n%128)*CH + n//128;
    # node NPAD-1 is the zero-gate sentinel (deg 1e30 -> norm ~ 0).
    pl.SENT = pl.NPAD - 1

    deg = np.bincount(dst, minlength=N).astype(np.int64)

    # snake assignment over degree-sorted nodes -> per-core node lists
    order = np.argsort(-deg, kind="stable")
    n8 = _ceil_to(N, ncores)
    order_p = np.concatenate([order, np.full(n8 - N, -1, dtype=np.int64)])
    blocks = order_p.reshape(-1, ncores).copy()
    blocks[1::2] = blocks[1::2, ::-1]
    core_nodes = np.ascontiguousarray(blocks.T)  # [ncores, npc]
    npc = core_nodes.shape[1]
    pl.NPC_PAD = _ceil_to(npc, P)
    pl.TILES = pl.NPC_PAD // P
    pad = np.full((ncores, pl.NPC_PAD - npc), -1, dtype=np.int64)
    core_nodes = np.concatenate([core_nodes, pad], axis=1)  # [ncores, NPC_PAD]
    pl.core_nodes = core_nodes

    node_deg = np.where(core_nodes >= 0, deg[np.clip(core_nodes, 0, N - 1)], 0)
    deg_tiles = node_deg.reshape(ncores, pl.TILES, P)
    Kt = deg_tiles.max(axis=(0, 2)).astype(np.int64)
    Kt = np.maximum(Kt, 1)
    pl.Kt = Kt
    pl.SX = int(Kt.sum())

    # CSR by dst
    e_order = np.argsort(dst, kind="stable")
    src_sorted = src[e_order]
    ends = np.cumsum(deg)
    starts = ends - deg

    CH = pl.CH

    def r_of(n):  # table row for node n (identity: partition p owns rows [p*CH,(p+1)*CH))
        return n

    # shared inputs
    xp = np.zeros((pl.NPAD, D), dtype=np.float32)
    xp[:N] = x
    wrep = np.empty((P, 128), dtype=np.float32)
    wrep[:, 0:64] = gate_w[0, 64:128][None, :]   # w_src
    wrep[:, 64:128] = gate_w[0, 0:64][None, :]   # w_dst
    b128 = np.full((P, 1), float(np.asarray(gate_b).reshape(-1)[0]), dtype=np.float32)
    degp = np.full(pl.NPAD, 1e30, dtype=np.float32)  # pad rows -> norm ~ 0
    degp[:N] = deg
    degt = np.ascontiguousarray(degp.reshape(P, CH))

    in_maps = []
    karange = np.arange(int(Kt.max()))[None, :]
    for c in range(ncores):
        # one slot-column stream per tile: [slot1..slotK], values are table
        # rows r(node) (partition-major layout)
        idx = np.full((P, pl.SX), pl.SENT, dtype=np.int32)
        koff = 0
        for t in range(pl.TILES):
            K = int(Kt[t])
            nodes = core_nodes[c, t * P : (t + 1) * P]  # [128]
            real = nodes >= 0
            d = np.where(real, deg[np.clip(nodes, 0, N - 1)], 0)
            st = np.where(real, starts[np.clip(nodes, 0, N - 1)], 0)
            mask = karange[:, :K] < d[:, None]  # [128, K]
            pos = st[:, None] + karange[:, :K]
            vals = src_sorted[np.minimum(pos, E - 1)]
            idx[:, koff : koff + K] = np.where(mask, r_of(vals), pl.SENT).astype(
                np.int32
            )
            koff += K
        # host-sharded owned rows in tile order (pads -> zero rows, deg 0)
        nodes_c = core_nodes[c]
        xown = np.zeros((pl.NPC_PAD, D), dtype=np.float32)
        realc = nodes_c >= 0
        xown[realc] = x[nodes_c[realc]]
        # partition-major [P, TILES*64] so the device load is contiguous per partition
        xown = np.ascontiguousarray(
            xown.reshape(pl.TILES, P, D).transpose(1, 0, 2).reshape(P, pl.TILES * D)
        )
        dgow_flat = np.where(realc, deg[np.clip(nodes_c, 0, N - 1)], 0).astype(
            np.float32
        )
        dgow = np.ascontiguousarray(dgow_flat.reshape(pl.TILES, P).T)  # [P, TILES]
        in_maps.append(
            {
                "xp": xp,
                "wrep": wrep,
                "b128": b128,
                "degt": degt,
                "idx": idx,
                "xown": xown,
                "dgow": dgow,
            }
        )
    return pl, in_maps


def _build_nc(pl):
    """Build the shared SPMD Bass/Tile program."""
    import concourse.bass as bass
    import concourse.bacc as bacc
    import concourse.mybir as mybir
    import concourse.tile as tile

    f32 = mybir.dt.float32
    bf16 = mybir.dt.bfloat16
    i32 = mybir.dt.int32
    AF = mybir.ActivationFunctionType
    OP = mybir.AluOpType

    D = pl.D
    CH = pl.CH
    TILES = pl.TILES
    Kt = [int(k) for k in pl.Kt]
    SX = pl.SX

    nc = bacc.Bacc("TRN2", target_bir_lowering=False, debug=False, num_devices=pl.ncores)
    xp_d = nc.dram_tensor("xp", [pl.NPAD, D], f32, kind="ExternalInput")
    wrep_d = nc.dram_tensor("wrep", [P, 128], f32, kind="ExternalInput")
    b128_d = nc.dram_tensor("b128", [P, 1], f32, kind="ExternalInput")
    degt_d = nc.dram_tensor("degt", [P, CH], f32, kind="ExternalInput")
    idx_d = nc.dram_tensor("idx", [P, SX], i32, kind="ExternalInput")
    xown_d = nc.dram_tensor("xown", [P, TILES * D], f32, kind="ExternalInput")
    dgow_d = nc.dram_tensor("dgow", [P, TILES], f32, kind="ExternalInput")
    z_d = nc.dram_tensor("z", [pl.NPC_PAD, D], f32, kind="ExternalOutput")
    # bf16 gather table, written once from the SBUF staging tile
    tab_d = nc.dram_tensor("tab", [pl.NPAD, FA], bf16)

    # batched phase-2 gathers: group tiles while sum(K) <= BATCH_K
    BATCH_K = 64
    batches = []
    b0 = 0
    while b0 < TILES:
        b1 = b0 + 1
        ks = Kt[b0]
        while b1 < TILES and ks + Kt[b1] <= BATCH_K:
            ks += Kt[b1]
            b1 += 1
        batches.append((b0, b1, ks))
        b0 = b1

    with tile.TileContext(nc) as tc:
        with (
            tc.tile_pool(name="consts", bufs=1) as cpool,
            tc.tile_pool(name="stage", bufs=1) as spool,
            tc.tile_pool(name="ph1", bufs=3) as p1pool,
            tc.tile_pool(name="gather", bufs=3) as gpool,
            tc.tile_pool(name="work", bufs=2) as wpool,
        ):
            idx_sb = cpool.tile([P, SX], i32)
            nc.sync.dma_start(out=idx_sb[:], in_=idx_d[:, :])
            wrep_sb = cpool.tile([P, 128], f32)
            nc.sync.dma_start(out=wrep_sb[:], in_=wrep_d[:, :])
            wsrcb = cpool.tile([P, 64], bf16)
            nc.vector.tensor_copy(out=wsrcb[:], in_=wrep_sb[:, 0:64])
            b128_sb = cpool.tile([P, 1], f32)
            nc.sync.dma_start(out=b128_sb[:], in_=b128_d[:, :])

            # ---- norms for all nodes: norm = rsqrt(max(deg, 1)) ----
            degt_sb = cpool.tile([P, CH], f32)
            nc.sync.dma_start(out=degt_sb[:], in_=degt_d[:, :])
            dclip = cpool.tile([P, CH], f32)
            nc.vector.tensor_scalar(
                out=dclip[:], in0=degt_sb[:], scalar1=1.0, scalar2=None, op0=OP.max
            )
            rec = cpool.tile([P, CH], f32)
            nc.vector.reciprocal(out=rec[:], in_=dclip[:])
            normT = cpool.tile([P, CH], f32)
            nc.scalar.activation(out=normT[:], in_=rec[:], func=AF.Sqrt)

            # ---- own-node scalars (p1b, norm) per tile, from host-sharded rows ----
            p1bT = cpool.tile([P, TILES], f32)
            normow = cpool.tile([P, TILES], f32)
            dgow_sb = cpool.tile([P, TILES], f32)
            nc.sync.dma_start(out=dgow_sb[:], in_=dgow_d[:, :])
            dgclip = cpool.tile([P, TILES], f32)
            nc.vector.tensor_scalar(
                out=dgclip[:], in0=dgow_sb[:], scalar1=1.0, scalar2=None, op0=OP.max
            )
            dgrec = cpool.tile([P, TILES], f32)
            nc.vector.reciprocal(out=dgrec[:], in_=dgclip[:])
            nc.scalar.activation(out=normow[:], in_=dgrec[:], func=AF.Sqrt)

            OB = 8  # own tiles per batch
            for t0 in range(0, TILES, OB):
                tn = min(OB, TILES - t0)
                xo = p1pool.tile([P, OB * 64], f32, tag="xo")
                xov = xo[:].rearrange("p (i f) -> p i f", f=64)
                nc.sync.dma_start(
                    out=xov[:, 0:tn, :],
                    in_=xown_d[:, t0 * 64 : (t0 + tn) * 64].rearrange(
                        "p (t f) -> p t f", f=64
                    ),
                )
                tmpo = p1pool.tile([P, OB * 64], f32, tag="tmpo")
                tov = tmpo[:].rearrange("p (i f) -> p i f", f=64)
                nc.vector.tensor_tensor(
                    out=tov[:, 0:tn, :],
                    in0=xov[:, 0:tn, :],
                    in1=wrep_sb[:, 64:128]
                    .rearrange("p (o f) -> p o f", o=1)
                    .to_broadcast([P, tn, 64]),
                    op=OP.mult,
                )
                redo = wpool.tile([P, OB], f32, tag="redo")
                nc.vector.tensor_reduce(
                    out=redo[:, 0:tn],
                    in_=tov[:, 0:tn, :],
                    axis=mybir.AxisListType.X,
                    op=OP.add,
                )
                nc.vector.tensor_scalar(
                    out=p1bT[:, t0 : t0 + tn],
                    in0=redo[:, 0:tn],
                    scalar1=b128_sb[:, 0:1],
                    scalar2=None,
                    op0=OP.add,
                )

            # ---- phase 1: bf16 table [x | p2, norm (f32 pairs) | pad] in SBUF ----
            ST = spool.tile([P, CH * FA], bf16)
            STv = ST[:].rearrange("p (c f) -> p c f", f=FA)
            STf = ST[:].bitcast(f32).rearrange("p (c q) -> p c q", q=FA32)
            tab_v = tab_d[0 : pl.NPAD, :].rearrange("(p c) f -> p c f", p=P)

            BC = 20  # chunks per batch
            half = ((CH // 2) + BC - 1) // BC * BC  # half boundary on a batch edge
            for c0 in range(0, CH, BC):
                cn = min(BC, CH - c0)
                xa = p1pool.tile([P, BC * 64], f32, tag="xa")
                xav = xa[:].rearrange("p (i f) -> p i f", f=64)
                nc.sync.dma_start(
                    out=xav[:, 0:cn, :],
                    in_=xp_d[0 : pl.NPAD, :].rearrange("(p c) f -> p c f", p=P)[
                        :, c0 : c0 + cn, :
                    ],
                )
                # x -> bf16 into the staging table (ACT engine; DVE is the
                # phase-1 critical path)
                nc.scalar.activation(
                    out=STv[:, c0 : c0 + cn, 0:64], in_=xav[:, 0:cn, :], func=AF.Copy
                )
                # p2 = x . w_src (bf16 multiply at 2x DVE rate, f32 reduce)
                tmp = p1pool.tile([P, BC * 64], bf16, tag="tmp")
                tv = tmp[:].rearrange("p (i f) -> p i f", f=64)
                nc.vector.tensor_tensor(
                    out=tv[:, 0:cn, :],
                    in0=STv[:, c0 : c0 + cn, 0:64],
                    in1=wsrcb[:]
                    .rearrange("p (o f) -> p o f", o=1)
                    .to_broadcast([P, cn, 64]),
                    op=OP.mult,
                )
                nc.vector.tensor_reduce(
                    out=STf[:, c0 : c0 + cn, COL_P2],
                    in_=tv[:, 0:cn, :],
                    axis=mybir.AxisListType.X,
                    op=OP.add,
                )
                nc.vector.tensor_copy(
                    out=STf[:, c0 : c0 + cn, COL_NORM], in_=normT[:, c0 : c0 + cn]
                )
                if c0 + cn == half:
                    nc.sync.dma_start(
                        out=tab_v[:, 0:half, :], in_=STv[:, 0:half, :]
                    )

            nc.sync.dma_start(out=tab_v[:, half:CH, :], in_=STv[:, half:CH, :])

            # ---- phase 2: gather + gate + aggregate ----
            coff = [0]
            for k in Kt:
                coff.append(coff[-1] + k)

            for b0, b1, ks in batches:
                s0 = coff[b0]
                ga = gpool.tile([P, (BATCH_K + 8) * FA], bf16, tag="ga")
                gav = ga[:].rearrange("p (k f) -> p k f", f=FA)
                gaf = ga[:].bitcast(f32).rearrange("p (k q) -> p k q", q=FA32)
                # one [P,1] indirect per slot column (only validated HW shape)
                for col in range(ks):
                    nc.gpsimd.indirect_dma_start(
                        out=ga[:, col * FA : (col + 1) * FA],
                        out_offset=None,
                        in_=tab_d[:, :],
                        in_offset=bass.IndirectOffsetOnAxis(
                            ap=idx_sb[:, s0 + col : s0 + col + 1], axis=0
                        ),
                    )
                koff = 0
                for t in range(b0, b1):
                    K = Kt[t]
                    xsl = gav[:, koff : koff + K, 0:64]
                    p2s = gaf[:, koff : koff + K, COL_P2]
                    nrs = gaf[:, koff : koff + K, COL_NORM]
                    # gate: tanh(p2_src + p1b_own) via ACT per-partition bias
                    tt = wpool.tile([P, K], f32, tag="tt")
                    nc.scalar.activation(
                        out=tt[:], in_=p2s, func=AF.Tanh, bias=p1bT[:, t : t + 1]
                    )
                    ee = wpool.tile([P, K], bf16, tag="ee")
                    nc.vector.tensor_tensor(out=ee[:], in0=tt[:], in1=nrs, op=OP.mult)
                    m = wpool.tile([P, K * 64], bf16, tag="m")
                    eev = (
                        ee[:]
                        .rearrange("p (k o) -> p k o", o=1)
                        .to_broadcast([P, K, 64])
                    )
                    nc.vector.tensor_tensor(
                        out=m[:].rearrange("p (k f) -> p k f", f=64),
                        in0=xsl,
                        in1=eev,
                        op=OP.mult,
                    )
                    red = wpool.tile([P, 64], f32, tag="red")
                    nc.vector.tensor_reduce(
                        out=red[:],
                        in_=m[:].rearrange("p (k f) -> p f k", f=64),
                        axis=mybir.AxisListType.X,
                        op=OP.add,
                    )
                    zt = wpool.tile([P, 64], f32, tag="zt")
                    nc.vector.tensor_scalar(
                        out=zt[:],
                        in0=red[:],
                        scalar1=normow[:, t : t + 1],
                        scalar2=None,
                        op0=OP.mult,
                    )
                    nc.sync.dma_start(out=z_d[t * P : (t + 1) * P, :], in_=zt[:])
                    koff += K
    nc.compile()
    return nc


_BUILD_CACHE = {}


def build(x, gate_w, gate_b, src, dst, ncores=8):
    pl, in_maps = _prep(x, gate_w, gate_b, src, dst, ncores)
    key = (pl.N, pl.E, pl.ncores, tuple(int(k) for k in pl.Kt))
    nc = _BUILD_CACHE.get(key)
    if nc is None:
        nc = _build_nc(pl)
        _BUILD_CACHE[key] = nc
    return pl, in_maps, nc


def _assemble(pl, outs):
    N, D = pl.N, pl.D
    z = np.zeros((N, D), dtype=np.float32)
    for c in range(pl.ncores):
        nodes = pl.core_nodes[c]
        real = nodes >= 0
        z[nodes[real]] = outs[c][real]
    return z


def kernel(x, gate_w, gate_b, src, dst):
    global LAST_RESULTS
    from concourse.bass_utils import run_bass_kernel_spmd

    pl, in_maps, nc = build(x, gate_w, gate_b, src, dst)
    res = run_bass_kernel_spmd(
        nc,
        in_maps,
        core_ids=list(range(pl.ncores)),
        trace=bool(int(os.environ.get("FAGCN_TRACE", "0"))),
    )
    LAST_RESULTS = res
    outs = [r["z"] for r in res.results]
    return _assemble(pl, outs)


# revision 13
# speedup vs baseline: 1.0007x; 1.0007x over previous
"""FAGCN message-passing kernel for 8 Trainium2 NeuronCores.

Strategy (edge-parallel via dst-ownership, v2):
  - Nodes are assigned to the 8 cores snake-wise in degree-sorted order, so
    every core owns ~N/8 nodes, ~E/8 edges, and sees the same degree profile
    (the compiled SPMD program is shared; only the index inputs differ).
  - Gate decomposition: tanh(Linear([h_dst, h_src])) = tanh(p1[dst] + p2[src] + b)
    with p1 = x @ w_dst, p2 = x @ w_src.
  - dst-side (own) scalars come from a host-sharded copy of the owned rows
    (xown, in tile order) -> p1b_own, norm_own per tile, no gather needed.
  - src-side: phase 1 builds a compact bf16 table [NPAD, 72] fully staged in
    SBUF: cols 0:64 = x (bf16), cols 64:68 = (p2, norm) as f32 (bitcast
    pairs), cols 68:72 pad.  One DMA (in two halves) writes it to DRAM.
    Rows are 144B, so phase-2 indirect gathers move ~half the bytes of the
    f32 variant.
  - Phase 2: per 128-node tile (dst-major, degree-sorted so slot padding is
    tiny), [P,1] indirect-DMA gathers pull table rows per slot column; the
    gate is ACT tanh with per-partition bias (p1b_own), aggregation is a
    bf16 DVE multiply + strided reduce (2x DVE rate).
    z[dst] = norm[dst] * sum_s tanh(p1b[dst] + p2[src]) * norm[src] * x[src].
"""

import os
import sys

sys.path.insert(0, "/opt/trn_rl_repo")

import numpy as np

P = 128
FA = 72  # bf16 elems per table row (144B): 64 x | 4 (=2 f32: p2, norm) | 4 pad
FA32 = FA // 2  # f32 view cols
COL_P2 = 32  # f32 col of p2
COL_NORM = 33  # f32 col of norm

LAST_RESULTS = None  # BassKernelResults of the most recent HW run (for profiling)


def _ceil_to(a, m):
    return ((a + m - 1) // m) * m


class Plan:
    pass


def _prep(x, gate_w, gate_b, src, dst, ncores=8):
    """Host-side sharding: shapes/constants + per-core input maps."""
    x = np.asarray(x, dtype=np.float32)
    gate_w = np.asarray(gate_w, dtype=np.float32)
    gate_b = np.asarray(gate_b, dtype=np.float32)
    src = np.asarray(src).astype(np.int64)
    dst = np.asarray(dst).astype(np.int64)

    N, D = x.shape
    assert D == 64
    E = src.shape[0]

    pl = Plan()
    pl.N, pl.D, pl.E, pl.ncores = N, D, E, ncores
    pl.NPAD = _ceil_to(N + 1, P)
    pl.CH = pl.NPAD // P
    # table row of node n under the partition-major layout r(n) = (n%128)*CH + n//128;
    # node NPAD-1 is the zero-gate sentinel (deg 1e30 -> norm ~ 0).
    pl.SENT = pl.NPAD - 1

    deg = np.bincount(dst, minlength=N).astype(np.int64)

    # snake assignment over degree-sorted nodes -> per-core node lists
    order = np.argsort(-deg, kind="stable")
    n8 = _ceil_to(N, ncores)
    order_p = np.concatenate([order, np.full(n8 - N, -1, dtype=np.int64)])
    blocks = order_p.reshape(-1, ncores).copy()
    blocks[1::2] = blocks[1::2, ::-1]
    core_nodes = np.ascontiguousarray(blocks.T)  # [ncores, npc]
    npc = core_nodes.shape[1]
    pl.NPC_PAD = _ceil_to(npc, P)
    pl.TILES = pl.NPC_PAD // P
    pad = np.full((ncores, pl.NPC_PAD - npc), -1, dtype=np.int64)
    core_nodes = np.concatenate([core_nodes, pad], axis=1)  # [ncores, NPC_PAD]
    pl.core_nodes = core_nodes

    node_deg = np.where(core_nodes >= 0, deg[np.clip(core_nodes, 0, N - 1)], 0)
    deg_tiles = node_deg.reshape(ncores, pl.TILES, P)
    Kt = deg_tiles.max(axis=(0, 2)).astype(np.int64)
    Kt = np.maximum(Kt, 1)
    pl.Kt = Kt
    pl.SX = int(Kt.sum())

    # CSR by dst
    e_order = np.argsort(dst, kind="stable")
    src_sorted = src[e_order]
    ends = np.cumsum(deg)
    starts = ends - deg

    CH = pl.CH

    def r_of(n):  # table row for node n (identity: partition p owns rows [p*CH,(p+1)*CH))
        return n

    # shared inputs
    xp = np.zeros((pl.NPAD, D), dtype=np.float32)
    xp[:N] = x
    wrep = np.empty((P, 128), dtype=np.float32)
    wrep[:, 0:64] = gate_w[0, 64:128][None, :]   # w_src
    wrep[:, 64:128] = gate_w[0, 0:64][None, :]   # w_dst
    b128 = np.full((P, 1), float(np.asarray(gate_b).reshape(-1)[0]), dtype=np.float32)
    degp = np.full(pl.NPAD, 1e30, dtype=np.float32)  # pad rows -> norm ~ 0
    degp[:N] = deg
    degt = np.ascontiguousarray(degp.reshape(P, CH))

    in_maps = []
    karange = np.arange(int(Kt.max()))[None, :]
    for c in range(ncores):
        # one slot-column stream per tile: [slot1..slotK], values are table
        # rows r(node) (partition-major layout)
        idx = np.full((P, pl.SX), pl.SENT, dtype=np.int32)
        koff = 0
        for t in range(pl.TILES):
            K = int(Kt[t])
            nodes = core_nodes[c, t * P : (t + 1) * P]  # [128]
            real = nodes >= 0
            d = np.where(real, deg[np.clip(nodes, 0, N - 1)], 0)
            st = np.where(real, starts[np.clip(nodes, 0, N - 1)], 0)
            mask = karange[:, :K] < d[:, None]  # [128, K]
            pos = st[:, None] + karange[:, :K]
            vals = src_sorted[np.minimum(pos, E - 1)]
            idx[:, koff : koff + K] = np.where(mask, r_of(vals), pl.SENT).astype(
                np.int32
            )
            koff += K
        # host-sharded owned rows in tile order (pads -> zero rows, deg 0)
        nodes_c = core_nodes[c]
        xown = np.zeros((pl.NPC_PAD, D), dtype=np.float32)
        realc = nodes_c >= 0
        xown[realc] = x[nodes_c[realc]]
        # partition-major [P, TILES*64] so the device load is contiguous per partition
        xown = np.ascontiguousarray(
            xown.reshape(pl.TILES, P, D).transpose(1, 0, 2).reshape(P, pl.TILES * D)
        )
        dgow_flat = np.where(realc, deg[np.clip(nodes_c, 0, N - 1)], 0).astype(
            np.float32
        )
        dgow = np.ascontiguousarray(dgow_flat.reshape(pl.TILES, P).T)  # [P, TILES]
        in_maps.append(
            {
                "xp": xp,
                "wrep": wrep,
                "b128": b128,
                "degt": degt,
                "idx": idx,
                "xown": xown,
                "dgow": dgow,
            }
        )
    return pl, in_maps


def _build_nc(pl):
    """Build the shared SPMD Bass/Tile program."""
    import concourse.bass as bass
    import concourse.bacc as bacc
    import concourse.mybir as mybir
    import concourse.tile as tile

    f32 = mybir.dt.float32
    bf16 = mybir.dt.bfloat16
    i32 = mybir.dt.int32
    AF = mybir.ActivationFunctionType
    OP = mybir.AluOpType

    D = pl.D
    CH = pl.CH
    TILES = pl.TILES
    Kt = [int(k) for k in pl.Kt]
    SX = pl.SX

    nc = bacc.Bacc("TRN2", target_bir_lowering=False, debug=False, num_devices=pl.ncores)
    xp_d = nc.dram_tensor("xp", [pl.NPAD, D], f32, kind="ExternalInput")
    wrep_d = nc.dram_tensor("wrep", [P, 128], f32, kind="ExternalInput")
    b128_d = nc.dram_tensor("b128", [P, 1], f32, kind="ExternalInput")
    degt_d = nc.dram_tensor("degt", [P, CH], f32, kind="ExternalInput")
    idx_d = nc.dram_tensor("idx", [P, SX], i32, kind="ExternalInput")
    xown_d = nc.dram_tensor("xown", [P, TILES * D], f32, kind="ExternalInput")
    dgow_d = nc.dram_tensor("dgow", [P, TILES], f32, kind="ExternalInput")
    z_d = nc.dram_tensor("z", [pl.NPC_PAD, D], f32, kind="ExternalOutput")
    # bf16 gather table, written once from the SBUF staging tile
    tab_d = nc.dram_tensor("tab", [pl.NPAD, FA], bf16)

    # batched phase-2 gathers: group tiles while sum(K) <= BATCH_K
    BATCH_K = 64
    batches = []
    b0 = 0
    while b0 < TILES:
        b1 = b0 + 1
        ks = Kt[b0]
        while b1 < TILES and ks + Kt[b1] <= BATCH_K:
            ks += Kt[b1]
            b1 += 1
        batches.append((b0, b1, ks))
        b0 = b1

    with tile.TileContext(nc) as tc:
        with (
            tc.tile_pool(name="consts", bufs=1) as cpool,
            tc.tile_pool(name="stage", bufs=1) as spool,
            tc.tile_pool(name="ph1", bufs=3) as p1pool,
            tc.tile_pool(name="gather", bufs=2) as gpool,
            tc.tile_pool(name="work", bufs=2) as wpool,
        ):
            idx_sb = cpool.tile([P, SX], i32)
            nc.sync.dma_start(out=idx_sb[:], in_=idx_d[:, :])
            wrep_sb = cpool.tile([P, 128], f32)
            nc.sync.dma_start(out=wrep_sb[:], in_=wrep_d[:, :])
            wsrcb = cpool.tile([P, 64], bf16)
            nc.vector.tensor_copy(out=wsrcb[:], in_=wrep_sb[:, 0:64])
            b128_sb = cpool.tile([P, 1], f32)
            nc.sync.dma_start(out=b128_sb[:], in_=b128_d[:, :])

            # ---- norms for all nodes: norm = rsqrt(max(deg, 1)) ----
            degt_sb = cpool.tile([P, CH], f32)
            nc.sync.dma_start(out=degt_sb[:], in_=degt_d[:, :])
            dclip = cpool.tile([P, CH], f32)
            nc.vector.tensor_scalar(
                out=dclip[:], in0=degt_sb[:], scalar1=1.0, scalar2=None, op0=OP.max
            )
            rec = cpool.tile([P, CH], f32)
            nc.vector.reciprocal(out=rec[:], in_=dclip[:])
            normT = cpool.tile([P, CH], f32)
            nc.scalar.activation(out=normT[:], in_=rec[:], func=AF.Sqrt)

            # ---- own-node scalars (p1b, norm) per tile, from host-sharded rows ----
            p1bT = cpool.tile([P, TILES], f32)
            normow = cpool.tile([P, TILES], f32)
            dgow_sb = cpool.tile([P, TILES], f32)
            nc.sync.dma_start(out=dgow_sb[:], in_=dgow_d[:, :])
            dgclip = cpool.tile([P, TILES], f32)
            nc.vector.tensor_scalar(
                out=dgclip[:], in0=dgow_sb[:], scalar1=1.0, scalar2=None, op0=OP.max
            )
            dgrec = cpool.tile([P, TILES], f32)
            nc.vector.reciprocal(out=dgrec[:], in_=dgclip[:])
            nc.scalar.activation(out=normow[:], in_=dgrec[:], func=AF.Sqrt)

            OB = 8  # own tiles per batch
            for t0 in range(0, TILES, OB):
                tn = min(OB, TILES - t0)
                xo = p1pool.tile([P, OB * 64], f32, tag="xo")
                xov = xo[:].rearrange("p (i f) -> p i f", f=64)
                nc.sync.dma_start(
                    out=xov[:, 0:tn, :],
                    in_=xown_d[:, t0 * 64 : (t0 + tn) * 64].rearrange(
                        "p (t f) -> p t f", f=64
                    ),
                )
                tmpo = p1pool.tile([P, OB * 64], f32, tag="tmpo")
                tov = tmpo[:].rearrange("p (i f) -> p i f", f=64)
                nc.vector.tensor_tensor(
                    out=tov[:, 0:tn, :],
                    in0=xov[:, 0:tn, :],
                    in1=wrep_sb[:, 64:128]
                    .rearrange("p (o f) -> p o f", o=1)
                    .to_broadcast([P, tn, 64]),
                    op=OP.mult,
                )
                redo = wpool.tile([P, OB], f32, tag="redo")
                nc.vector.tensor_reduce(
                    out=redo[:, 0:tn],
                    in_=tov[:, 0:tn, :],
                    axis=mybir.AxisListType.X,
                    op=OP.add,
                )
                nc.vector.tensor_scalar(
                    out=p1bT[:, t0 : t0 + tn],
                    in0=redo[:, 0:tn],
                    scalar1=b128_sb[:, 0:1],
                    scalar2=None,
                    op0=OP.add,
                )

            # ---- phase 1: bf16 table [x | p2, norm (f32 pairs) | pad] in SBUF ----
            ST = spool.tile([P, CH * FA], bf16)
            STv = ST[:].rearrange("p (c f) -> p c f", f=FA)
            STf = ST[:].bitcast(f32).rearrange("p (c q) -> p c q", q=FA32)
            tab_v = tab_d[0 : pl.NPAD, :].rearrange("(p c) f -> p c f", p=P)

            BC = 20  # chunks per batch
            half = ((CH // 2) + BC - 1) // BC * BC  # half boundary on a batch edge
            for c0 in range(0, CH, BC):
                cn = min(BC, CH - c0)
                xa = p1pool.tile([P, BC * 64], f32, tag="xa")
                xav = xa[:].rearrange("p (i f) -> p i f", f=64)
                nc.sync.dma_start(
                    out=xav[:, 0:cn, :],
                    in_=xp_d[0 : pl.NPAD, :].rearrange("(p c) f -> p c f", p=P)[
                        :, c0 : c0 + cn, :
                    ],
                )
                # x -> bf16 into the staging table (ACT engine; DVE is the
                # phase-1 critical path)
                nc.scalar.activation(
                    out=STv[:, c0 : c0 + cn, 0:64], in_=xav[:, 0:cn, :], func=AF.Copy
                )
                # p2 = x . w_src (bf16 multiply at 2x DVE rate, f32 reduce)
                tmp = p1pool.tile([P, BC * 64], bf16, tag="tmp")
                tv = tmp[:].rearrange("p (i f) -> p i f", f=64)
                nc.vector.tensor_tensor(
                    out=tv[:, 0:cn, :],
                    in0=STv[:, c0 : c0 + cn, 0:64],
                    in1=wsrcb[:]
                    .rearrange("p (o f) -> p o f", o=1)
                    .to_broadcast([P, cn, 64]),
                    op=OP.mult,
                )
                nc.vector.tensor_reduce(
                    out=STf[:, c0 : c0 + cn, COL_P2],
                    in_=tv[:, 0:cn, :],
                    axis=mybir.AxisListType.X,
                    op=OP.add,
                )
                nc.vector.tensor_copy(
                    out=STf[:, c0 : c0 + cn, COL_NORM], in_=normT[:, c0 : c0 + cn]
                )
                if c0 + cn == half:
                    nc.sync.dma_start(
                        out=tab_v[:, 0:half, :], in_=STv[:, 0:half, :]
                    )

            nc.sync.dma_start(out=tab_v[:, half:CH, :], in_=STv[:, half:CH, :])

            # ---- phase 2: gather + gate + aggregate ----
            coff = [0]
            for k in Kt:
                coff.append(coff[-1] + k)

            for b0, b1, ks in batches:
                s0 = coff[b0]
                ga = gpool.tile([P, (BATCH_K + 8) * FA], bf16, tag="ga")
                gav = ga[:].rearrange("p (k f) -> p k f", f=FA)
                gaf = ga[:].bitcast(f32).rearrange("p (k q) -> p k q", q=FA32)
                # one [P,1] indirect per slot column (only validated HW shape)
                for col in range(ks):
                    nc.gpsimd.indirect_dma_start(
                        out=ga[:, col * FA : (col + 1) * FA],
                        out_offset=None,
                        in_=tab_d[:, :],
                        in_offset=bass.IndirectOffsetOnAxis(
                            ap=idx_sb[:, s0 + col : s0 + col + 1], axis=0
                        ),
                    )
                koff = 0
                for t in range(b0, b1):
                    K = Kt[t]
                    xsl = gav[:, koff : koff + K, 0:64]
                    p2s = gaf[:, koff : koff + K, COL_P2]
                    nrs = gaf[:, koff : koff + K, COL_NORM]
                    # gate: tanh(p2_src + p1b_own) via ACT per-partition bias
                    tt = wpool.tile([P, K], f32, tag="tt")
                    nc.scalar.activation(
                        out=tt[:], in_=p2s, func=AF.Tanh, bias=p1bT[:, t : t + 1]
                    )
                    ee = wpool.tile([P, K], bf16, tag="ee")
                    nc.vector.tensor_tensor(out=ee[:], in0=tt[:], in1=nrs, op=OP.mult)
                    m = wpool.tile([P, K * 64], bf16, tag="m")
                    eev = (
                        ee[:]
                        .rearrange("p (k o) -> p k o", o=1)
                        .to_broadcast([P, K, 64])
                    )
                    nc.vector.tensor_tensor(
                        out=m[:].rearrange("p (k f) -> p k f", f=64),
                        in0=xsl,
                        in1=eev,
                        op=OP.mult,
                    )
                    red = wpool.tile([P, 64], f32, tag="red")
                    nc.vector.tensor_reduce(
                        out=red[:],
                        in_=m[:].rearrange("p (k f) -> p f k", f=64),
                        axis=mybir.AxisListType.X,
                        op=OP.add,
                    )
                    zt = wpool.tile([P, 64], f32, tag="zt")
                    nc.vector.tensor_scalar(
                        out=zt[:],
                        in0=red[:],
                        scalar1=normow[:, t : t + 1],
                        scalar2=None,
                        op0=OP.mult,
                    )
                    nc.sync.dma_start(out=z_d[t * P : (t + 1) * P, :], in_=zt[:])
                    koff += K
    nc.compile()
    return nc


_BUILD_CACHE = {}


def build(x, gate_w, gate_b, src, dst, ncores=8):
    pl, in_maps = _prep(x, gate_w, gate_b, src, dst, ncores)
    key = (pl.N, pl.E, pl.ncores, tuple(int(k) for k in pl.Kt))
    nc = _BUILD_CACHE.get(key)
    if nc is None:
        nc = _build_nc(pl)
        _BUILD_CACHE[key] = nc
    return pl, in_maps, nc


def _assemble(pl, outs):
    N, D = pl.N, pl.D
    z = np.zeros((N, D), dtype=np.float32)
    for c in range(pl.ncores):
        nodes = pl.core_nodes[c]
        real = nodes >= 0
        z[nodes[real]] = outs[c][real]
    return z


def kernel(x, gate_w, gate_b, src, dst):
    global LAST_RESULTS
    from concourse.bass_utils import run_bass_kernel_spmd

    pl, in_maps, nc = build(x, gate_w, gate_b, src, dst)
    res = run_bass_kernel_spmd(
        nc,
        in_maps,
        core_ids=list(range(pl.ncores)),
        trace=bool(int(os.environ.get("FAGCN_TRACE", "0"))),
    )
    LAST_RESULTS = res
    outs = [r["z"] for r in res.results]
    return _assemble(pl, outs)


# revision 14
# speedup vs baseline: 1.0420x; 1.0412x over previous
"""FAGCN message-passing kernel for 8 Trainium2 NeuronCores.

Strategy (edge-parallel via dst-ownership, v4 — tableless):
  - Nodes are assigned to the 8 cores snake-wise in degree-sorted order, so
    every core owns ~N/8 nodes, ~E/8 edges, and sees the same degree profile
    (the compiled SPMD program is shared; only the index inputs differ).
  - Gate decomposition: tanh(Linear([h_dst, h_src])) = tanh(p1[dst] + p2[src] + b)
    with p1 = x @ w_dst, p2 = x @ w_src.
  - Phase-2 indirect gathers read raw f32 x rows STRAIGHT from the input
    tensor (no device-built table, so gathers start immediately); p2 is
    computed per-edge on the DVE, which hides entirely under the gather
    span (the [P,1] indirect-DMA issue rate on GpSimd is the wall).
  - norm[src] is a pure function of in-degrees (index data), so the host
    ships it pre-expanded in slot layout (normsl), like idx.
  - dst-side (own) scalars come from a host-sharded copy of the owned rows
    (xown, in tile order) -> p1b_own, norm_own per tile; this small DVE
    block overlaps the first gather batch.
  - Per 128-node tile (dst-major, degree-sorted so slot padding is tiny):
    gate is ACT tanh with per-partition bias (p1b_own);
    z[dst] = norm[dst] * sum_s tanh(p1b[dst] + p2[src]) * norm[src] * x[src].
"""

import os
import sys

sys.path.insert(0, "/opt/trn_rl_repo")

import numpy as np

P = 128
D64 = 64

LAST_RESULTS = None  # BassKernelResults of the most recent HW run (for profiling)


def _ceil_to(a, m):
    return ((a + m - 1) // m) * m


class Plan:
    pass


def _prep(x, gate_w, gate_b, src, dst, ncores=8):
    """Host-side sharding: shapes/constants + per-core input maps."""
    x = np.asarray(x, dtype=np.float32)
    gate_w = np.asarray(gate_w, dtype=np.float32)
    gate_b = np.asarray(gate_b, dtype=np.float32)
    src = np.asarray(src).astype(np.int64)
    dst = np.asarray(dst).astype(np.int64)

    N, D = x.shape
    assert D == 64
    E = src.shape[0]

    pl = Plan()
    pl.N, pl.D, pl.E, pl.ncores = N, D, E, ncores
    pl.NPAD = _ceil_to(N + 1, P)
    # sentinel row: x = 0, norm slot = 0 -> zero contribution
    pl.SENT = pl.NPAD - 1

    deg = np.bincount(dst, minlength=N).astype(np.int64)

    # snake assignment over degree-sorted nodes -> per-core node lists
    order = np.argsort(-deg, kind="stable")
    n8 = _ceil_to(N, ncores)
    order_p = np.concatenate([order, np.full(n8 - N, -1, dtype=np.int64)])
    blocks = order_p.reshape(-1, ncores).copy()
    blocks[1::2] = blocks[1::2, ::-1]
    core_nodes = np.ascontiguousarray(blocks.T)  # [ncores, npc]
    npc = core_nodes.shape[1]
    pl.NPC_PAD = _ceil_to(npc, P)
    pl.TILES = pl.NPC_PAD // P
    pad = np.full((ncores, pl.NPC_PAD - npc), -1, dtype=np.int64)
    core_nodes = np.concatenate([core_nodes, pad], axis=1)  # [ncores, NPC_PAD]
    pl.core_nodes = core_nodes

    node_deg = np.where(core_nodes >= 0, deg[np.clip(core_nodes, 0, N - 1)], 0)
    deg_tiles = node_deg.reshape(ncores, pl.TILES, P)
    Kt = deg_tiles.max(axis=(0, 2)).astype(np.int64)
    Kt = np.maximum(Kt, 1)
    pl.Kt = Kt
    pl.SX = int(Kt.sum())

    # CSR by dst
    e_order = np.argsort(dst, kind="stable")
    src_sorted = src[e_order]
    ends = np.cumsum(deg)
    starts = ends - deg

    # shared inputs
    xp = np.zeros((pl.NPAD, D), dtype=np.float32)
    xp[:N] = x
    wrep = np.empty((P, 128), dtype=np.float32)
    wrep[:, 0:64] = gate_w[0, 64:128][None, :]   # w_src
    wrep[:, 64:128] = gate_w[0, 0:64][None, :]   # w_dst
    b128 = np.full((P, 1), float(np.asarray(gate_b).reshape(-1)[0]), dtype=np.float32)
    normv = (1.0 / np.sqrt(np.maximum(deg, 1))).astype(np.float32)  # [N]

    in_maps = []
    karange = np.arange(int(Kt.max()))[None, :]
    for c in range(ncores):
        # one slot-column stream per tile: [slot1..slotK]; idx = x row of the
        # source node, normsl = norm[source] (0 on sentinel slots)
        idx = np.full((P, pl.SX), pl.SENT, dtype=np.int32)
        normsl = np.zeros((P, pl.SX), dtype=np.float32)
        koff = 0
        for t in range(pl.TILES):
            K = int(Kt[t])
            nodes = core_nodes[c, t * P : (t + 1) * P]  # [128]
            real = nodes >= 0
            d = np.where(real, deg[np.clip(nodes, 0, N - 1)], 0)
            st = np.where(real, starts[np.clip(nodes, 0, N - 1)], 0)
            mask = karange[:, :K] < d[:, None]  # [128, K]
            pos = st[:, None] + karange[:, :K]
            vals = src_sorted[np.minimum(pos, E - 1)]
            idx[:, koff : koff + K] = np.where(mask, vals, pl.SENT).astype(np.int32)
            normsl[:, koff : koff + K] = np.where(mask, normv[vals], 0.0).astype(
                np.float32
            )
            koff += K
        # host-sharded owned rows, partition-major [P, TILES*64] (contiguous load)
        nodes_c = core_nodes[c]
        xown = np.zeros((pl.NPC_PAD, D), dtype=np.float32)
        realc = nodes_c >= 0
        xown[realc] = x[nodes_c[realc]]
        xown = np.ascontiguousarray(
            xown.reshape(pl.TILES, P, D).transpose(1, 0, 2).reshape(P, pl.TILES * D)
        )
        dgow_flat = np.where(realc, deg[np.clip(nodes_c, 0, N - 1)], 0).astype(
            np.float32
        )
        dgow = np.ascontiguousarray(dgow_flat.reshape(pl.TILES, P).T)  # [P, TILES]
        in_maps.append(
            {
                "xp": xp,
                "wrep": wrep,
                "b128": b128,
                "idx": idx,
                "normsl": normsl,
                "xown": xown,
                "dgow": dgow,
            }
        )
    return pl, in_maps


def _build_nc(pl):
    """Build the shared SPMD Bass/Tile program."""
    import concourse.bass as bass
    import concourse.bacc as bacc
    import concourse.mybir as mybir
    import concourse.tile as tile

    f32 = mybir.dt.float32
    i32 = mybir.dt.int32
    AF = mybir.ActivationFunctionType
    OP = mybir.AluOpType

    D = pl.D
    TILES = pl.TILES
    Kt = [int(k) for k in pl.Kt]
    SX = pl.SX

    nc = bacc.Bacc("TRN2", target_bir_lowering=False, debug=False, num_devices=pl.ncores)
    xp_d = nc.dram_tensor("xp", [pl.NPAD, D], f32, kind="ExternalInput")
    wrep_d = nc.dram_tensor("wrep", [P, 128], f32, kind="ExternalInput")
    b128_d = nc.dram_tensor("b128", [P, 1], f32, kind="ExternalInput")
    idx_d = nc.dram_tensor("idx", [P, SX], i32, kind="ExternalInput")
    normsl_d = nc.dram_tensor("normsl", [P, SX], f32, kind="ExternalInput")
    xown_d = nc.dram_tensor("xown", [P, TILES * D], f32, kind="ExternalInput")
    dgow_d = nc.dram_tensor("dgow", [P, TILES], f32, kind="ExternalInput")
    z_d = nc.dram_tensor("z", [pl.NPC_PAD, D], f32, kind="ExternalOutput")

    # batched phase-2 gathers: group tiles while sum(K) <= BATCH_K
    BATCH_K = 64
    batches = []
    b0 = 0
    while b0 < TILES:
        b1 = b0 + 1
        ks = Kt[b0]
        while b1 < TILES and ks + Kt[b1] <= BATCH_K:
            ks += Kt[b1]
            b1 += 1
        batches.append((b0, b1, ks))
        b0 = b1

    with tile.TileContext(nc) as tc:
        with (
            tc.tile_pool(name="consts", bufs=1) as cpool,
            tc.tile_pool(name="own", bufs=3) as p1pool,
            tc.tile_pool(name="gather", bufs=2) as gpool,
            tc.tile_pool(name="work", bufs=2) as wpool,
        ):
            # gathers depend only on idx — load it first so they start at once
            idx_sb = cpool.tile([P, SX], i32)
            nc.sync.dma_start(out=idx_sb[:], in_=idx_d[:, :])
            normsl_sb = cpool.tile([P, SX], f32)
            nc.sync.dma_start(out=normsl_sb[:], in_=normsl_d[:, :])
            wrep_sb = cpool.tile([P, 128], f32)
            nc.sync.dma_start(out=wrep_sb[:], in_=wrep_d[:, :])
            b128_sb = cpool.tile([P, 1], f32)
            nc.sync.dma_start(out=b128_sb[:], in_=b128_d[:, :])

            # ---- own-node scalars (p1b, norm) per tile; overlaps first gathers ----
            p1bT = cpool.tile([P, TILES], f32)
            normow = cpool.tile([P, TILES], f32)
            dgow_sb = cpool.tile([P, TILES], f32)
            nc.sync.dma_start(out=dgow_sb[:], in_=dgow_d[:, :])
            dgclip = cpool.tile([P, TILES], f32)
            nc.vector.tensor_scalar(
                out=dgclip[:], in0=dgow_sb[:], scalar1=1.0, scalar2=None, op0=OP.max
            )
            dgrec = cpool.tile([P, TILES], f32)
            nc.vector.reciprocal(out=dgrec[:], in_=dgclip[:])
            nc.scalar.activation(out=normow[:], in_=dgrec[:], func=AF.Sqrt)

            OB = 8
            for t0 in range(0, TILES, OB):
                tn = min(OB, TILES - t0)
                xo = p1pool.tile([P, OB * 64], f32, tag="xo")
                xov = xo[:].rearrange("p (i f) -> p i f", f=64)
                nc.sync.dma_start(
                    out=xov[:, 0:tn, :],
                    in_=xown_d[:, t0 * 64 : (t0 + tn) * 64].rearrange(
                        "p (t f) -> p t f", f=64
                    ),
                )
                tmpo = p1pool.tile([P, OB * 64], f32, tag="tmpo")
                tov = tmpo[:].rearrange("p (i f) -> p i f", f=64)
                nc.vector.tensor_tensor(
                    out=tov[:, 0:tn, :],
                    in0=xov[:, 0:tn, :],
                    in1=wrep_sb[:, 64:128]
                    .rearrange("p (o f) -> p o f", o=1)
                    .to_broadcast([P, tn, 64]),
                    op=OP.mult,
                )
                redo = wpool.tile([P, OB], f32, tag="redo")
                nc.vector.tensor_reduce(
                    out=redo[:, 0:tn],
                    in_=tov[:, 0:tn, :],
                    axis=mybir.AxisListType.X,
                    op=OP.add,
                )
                nc.vector.tensor_scalar(
                    out=p1bT[:, t0 : t0 + tn],
                    in0=redo[:, 0:tn],
                    scalar1=b128_sb[:, 0:1],
                    scalar2=None,
                    op0=OP.add,
                )

            # ---- phase 2: gather raw x rows + per-edge gate + aggregate ----
            coff = [0]
            for k in Kt:
                coff.append(coff[-1] + k)

            for b0, b1, ks in batches:
                s0 = coff[b0]
                ga = gpool.tile([P, (BATCH_K + 8) * D64], f32, tag="ga")
                gav = ga[:].rearrange("p (k f) -> p k f", f=D64)
                # one [P,1] indirect per slot column (only validated HW shape)
                for col in range(ks):
                    nc.gpsimd.indirect_dma_start(
                        out=ga[:, col * D64 : (col + 1) * D64],
                        out_offset=None,
                        in_=xp_d[:, :],
                        in_offset=bass.IndirectOffsetOnAxis(
                            ap=idx_sb[:, s0 + col : s0 + col + 1], axis=0
                        ),
                    )
                koff = 0
                for t in range(b0, b1):
                    K = Kt[t]
                    xsl = gav[:, koff : koff + K, :]
                    # p2 = x_src . w_src per edge
                    t2 = wpool.tile([P, K * 64], f32, tag="t2")
                    nc.vector.tensor_tensor(
                        out=t2[:].rearrange("p (k f) -> p k f", f=64),
                        in0=xsl,
                        in1=wrep_sb[:, 0:64]
                        .rearrange("p (o f) -> p o f", o=1)
                        .to_broadcast([P, K, 64]),
                        op=OP.mult,
                    )
                    p2d = wpool.tile([P, K], f32, tag="p2d")
                    nc.vector.tensor_reduce(
                        out=p2d[:],
                        in_=t2[:].rearrange("p (k f) -> p k f", f=64),
                        axis=mybir.AxisListType.X,
                        op=OP.add,
                    )
                    tt = wpool.tile([P, K], f32, tag="tt")
                    nc.scalar.activation(
                        out=tt[:], in_=p2d[:], func=AF.Tanh, bias=p1bT[:, t : t + 1]
                    )
                    ee = wpool.tile([P, K], f32, tag="ee")
                    nc.vector.tensor_tensor(
                        out=ee[:],
                        in0=tt[:],
                        in1=normsl_sb[:, s0 + koff : s0 + koff + K],
                        op=OP.mult,
                    )
                    m = wpool.tile([P, K * 64], f32, tag="m")
                    eev = (
                        ee[:]
                        .rearrange("p (k o) -> p k o", o=1)
                        .to_broadcast([P, K, 64])
                    )
                    nc.vector.tensor_tensor(
                        out=m[:].rearrange("p (k f) -> p k f", f=64),
                        in0=xsl,
                        in1=eev,
                        op=OP.mult,
                    )
                    red = wpool.tile([P, 64], f32, tag="red")
                    nc.vector.tensor_reduce(
                        out=red[:],
                        in_=m[:].rearrange("p (k f) -> p f k", f=64),
                        axis=mybir.AxisListType.X,
                        op=OP.add,
                    )
                    zt = wpool.tile([P, 64], f32, tag="zt")
                    nc.vector.tensor_scalar(
                        out=zt[:],
                        in0=red[:],
                        scalar1=normow[:, t : t + 1],
                        scalar2=None,
                        op0=OP.mult,
                    )
                    nc.sync.dma_start(out=z_d[t * P : (t + 1) * P, :], in_=zt[:])
                    koff += K
    nc.compile()
    return nc


_BUILD_CACHE = {}


def build(x, gate_w, gate_b, src, dst, ncores=8):
    pl, in_maps = _prep(x, gate_w, gate_b, src, dst, ncores)
    key = (pl.N, pl.E, pl.ncores, tuple(int(k) for k in pl.Kt))
    nc = _BUILD_CACHE.get(key)
    if nc is None:
        nc = _build_nc(pl)
        _BUILD_CACHE[key] = nc
    return pl, in_maps, nc


def _assemble(pl, outs):
    N, D = pl.N, pl.D
    z = np.zeros((N, D), dtype=np.float32)
    for c in range(pl.ncores):
        nodes = pl.core_nodes[c]
        real = nodes >= 0
        z[nodes[real]] = outs[c][real]
    return z


def kernel(x, gate_w, gate_b, src, dst):
    global LAST_RESULTS
    from concourse.bass_utils import run_bass_kernel_spmd

    pl, in_maps, nc = build(x, gate_w, gate_b, src, dst)
    res = run_bass_kernel_spmd(
        nc,
        in_maps,
        core_ids=list(range(pl.ncores)),
        trace=bool(int(os.environ.get("FAGCN_TRACE", "0"))),
    )
    LAST_RESULTS = res
    outs = [r["z"] for r in res.results]
    return _assemble(pl, outs)
